# Initial kernel scaffold
#
"""EpisodicSlotWriter Trainium2 kernel.

Math (forward values only — the straight-through trick makes write_w equal the
hard one-hot of argmax(sim)):
  sim[b,k]   = dot(epi_keys[b,k], wk_n[b]) / (||epi_keys[b,k]|| + EPS)
  top[b]     = argmax_k sim[b,k];  best[b] = max_k sim[b,k]
  keys_new   = normalize(epi_keys) rows, except row top[b] which is the blended
               row (1-r)*epi_keys + r*write_key, normalized; r = 0.5*clip(ws,0,1)
  vals_new   = epi_vals, except row top[b] blended the same way
  age_new    = (epi_age+1) zeroed at top[b]
  str_new    = clip(0.995*str [+ ws*(1-0.995*str) at top[b]], 0.001, 1.0)

Device (8 NeuronCores, batch-parallel, 8 batches/core) streams the heavy
tensors: normalizes every epi_keys row + computes sim (the full 256MB keys
in/out), and copies epi_vals to vals_new (256MB in/out) via DRAM->DRAM DMA.
Host does the O(B*D) fix-up of the single written row per batch plus the tiny
(B,K) age/strength maps.
"""

import os
from contextlib import ExitStack

import numpy as np

EPS = 1e-6
B, K, D = 64, 2048, 512
M = 8            # cores
BPC = B // M     # batches per core
P = 128          # partitions
TPB = K // P     # k-tiles per batch

_PROGRAM = None
LAST_RESULT = None


def _build_program():
    import concourse.bacc as bacc
    import concourse.mybir as mybir
    import concourse.tile as tile

    f32 = mybir.dt.float32
    nc = bacc.Bacc(
        "TRN2",
        target_bir_lowering=False,
        debug=False,
        num_devices=M,
    )

    keys_in = nc.dram_tensor("keys_in", [BPC * K, D], f32, kind="ExternalInput").ap()
    vals_in = nc.dram_tensor("vals_in", [BPC * K, D], f32, kind="ExternalInput").ap()
    wbc = nc.dram_tensor("wbc", [BPC, P, D], f32, kind="ExternalInput").ap()
    keys_out = nc.dram_tensor("keys_out", [BPC * K, D], f32, kind="ExternalOutput").ap()
    vals_out = nc.dram_tensor("vals_out", [BPC * K, D], f32, kind="ExternalOutput").ap()
    sim_out = nc.dram_tensor("sim_out", [P, BPC * TPB], f32, kind="ExternalOutput").ap()

    with tile.TileContext(nc) as tc:
        with ExitStack() as ctx:
            kpool = ctx.enter_context(tc.tile_pool(name="kpool", bufs=24))
            spool = ctx.enter_context(tc.tile_pool(name="spool", bufs=3))
            wpool = ctx.enter_context(tc.tile_pool(name="wpool", bufs=2))
            cpool = ctx.enter_context(tc.tile_pool(name="cpool", bufs=3))
            simpool = ctx.enter_context(tc.tile_pool(name="simpool", bufs=1))

            # Pass-through copy of epi_vals, DRAM->DRAM, in 4MB chunks.
            n_chunks = 8
            rpc = BPC * K // n_chunks
            for c in range(n_chunks):
                nc.sync.dma_start(
                    out=vals_out[c * rpc : (c + 1) * rpc, :],
                    in_=vals_in[c * rpc : (c + 1) * rpc, :],
                )

            sim_sb = simpool.tile([P, BPC * TPB], f32, name="sim_sb", tag="sim_sb")

            for b in range(BPC):
                wt = wpool.tile([P, D], f32, name="wt", tag="wt")
                nc.sync.dma_start(wt[:], wbc[b])
                n2 = cpool.tile([P, TPB], f32, name="n2", tag="n2")
                dot = cpool.tile([P, TPB], f32, name="dot", tag="dot")
                kts = []
                for t in range(TPB):
                    r0 = b * K + t * P
                    kt = kpool.tile([P, D], f32, name="kt", tag="kt")
                    nc.sync.dma_start(kt[:], keys_in[r0 : r0 + P, :])
                    # ACT: norm2 per row (square + accumulate along free axis)
                    sq = spool.tile([P, D], f32, name="sq", tag="sq")
                    nc.scalar.activation(
                        sq[:],
                        kt[:],
                        mybir.ActivationFunctionType.Square,
                        accum_out=n2[:, t : t + 1],
                    )
                    # DVE: dot per row with broadcast write-key
                    prod = spool.tile([P, D], f32, name="prod", tag="prod")
                    nc.vector.tensor_tensor_reduce(
                        out=prod[:],
                        in0=kt[:],
                        in1=wt[:],
                        scale=1.0,
                        scalar=0.0,
                        op0=mybir.AluOpType.mult,
                        op1=mybir.AluOpType.add,
                        accum_out=dot[:, t : t + 1],
                    )
                    kts.append((kt, r0))

                # inv = 1 / (sqrt(n2) + EPS), batched over the 16 tile columns
                sg = cpool.tile([P, TPB], f32, name="sg", tag="sg")
                nc.scalar.activation(sg[:], n2[:], mybir.ActivationFunctionType.Sqrt)
                nc.vector.tensor_scalar_add(sg[:], sg[:], EPS)
                inv = cpool.tile([P, TPB], f32, name="inv", tag="inv")
                nc.vector.reciprocal(inv[:], sg[:])
                nc.vector.tensor_mul(
                    sim_sb[:, b * TPB : (b + 1) * TPB], dot[:], inv[:]
                )

                for t, (kt, r0) in enumerate(kts):
                    nc.vector.tensor_scalar_mul(kt[:], kt[:], inv[:, t : t + 1])
                    nc.sync.dma_start(keys_out[r0 : r0 + P, :], kt[:])

            nc.sync.dma_start(sim_out[:], sim_sb[:])

    nc.compile()
    return nc


def _get_program():
    global _PROGRAM
    if _PROGRAM is None:
        _PROGRAM = _build_program()
    return _PROGRAM


def kernel(**inputs):
    global LAST_RESULT
    from concourse.bass_utils import run_bass_kernel_spmd

    wk = np.asarray(inputs["write_key"], dtype=np.float32)
    wv = np.asarray(inputs["write_val"], dtype=np.float32)
    ws_raw = np.asarray(inputs["write_strength"], dtype=np.float32)
    ek = np.ascontiguousarray(np.asarray(inputs["epi_keys"], dtype=np.float32))
    ev = np.ascontiguousarray(np.asarray(inputs["epi_vals"], dtype=np.float32))
    ea = np.asarray(inputs["epi_age"], dtype=np.float32)
    es = np.asarray(inputs["epi_strength"], dtype=np.float32)

    nc = _get_program()

    wk_n = wk / (np.linalg.norm(wk, axis=-1, keepdims=True) + np.float32(EPS))
    in_maps = []
    for c in range(M):
        sl = slice(c * BPC, (c + 1) * BPC)
        in_maps.append(
            {
                "keys_in": ek[sl].reshape(BPC * K, D),
                "vals_in": ev[sl].reshape(BPC * K, D),
                "wbc": np.ascontiguousarray(
                    np.broadcast_to(wk_n[sl][:, None, :], (BPC, P, D))
                ),
            }
        )

    trace = bool(int(os.environ.get("KERNEL_TRACE", "0")))
    res = run_bass_kernel_spmd(nc, in_maps, list(range(M)), trace=trace)
    LAST_RESULT = res

    keys_new = np.empty((B, K, D), dtype=np.float32)
    vals_new = np.empty((B, K, D), dtype=np.float32)
    sim = np.empty((B, K), dtype=np.float32)
    for c in range(M):
        sl = slice(c * BPC, (c + 1) * BPC)
        keys_new[sl] = res.results[c]["keys_out"].reshape(BPC, K, D)
        vals_new[sl] = res.results[c]["vals_out"].reshape(BPC, K, D)
        # sim_out is [p, b*TPB + t] with k = t*P + p
        sim_raw = res.results[c]["sim_out"].reshape(P, BPC, TPB)
        sim[sl] = sim_raw.transpose(1, 2, 0).reshape(BPC, K)

    top = sim.argmax(axis=1)
    best_sim = sim.max(axis=1)
    slot_idx = top.astype(np.int32)

    ws = np.clip(ws_raw, 0.0, 1.0)
    r = ws * np.float32(0.5)  # (B,)
    ar = np.arange(B)

    # fix the written row of keys (blend then normalize)
    kb = (1.0 - r)[:, None] * ek[ar, top] + r[:, None] * wk
    kbn = kb / (np.linalg.norm(kb, axis=-1, keepdims=True) + np.float32(EPS))
    keys_new[ar, top] = kbn

    # fix the written row of vals
    vals_new[ar, top] = (1.0 - r)[:, None] * ev[ar, top] + r[:, None] * wv

    age_new = ea + np.float32(1.0)
    age_new[ar, top] = 0.0

    s0 = es * np.float32(0.995)
    str_new = np.clip(s0, 0.001, 1.0).astype(np.float32)
    str_new[ar, top] = np.clip(
        s0[ar, top] + ws * (np.float32(1.0) - s0[ar, top]), 0.001, 1.0
    )

    return keys_new, vals_new, age_new, str_new, slot_idx, best_sim


# revision 5
# speedup vs baseline: 1.5339x; 1.5339x over previous
"""EpisodicSlotWriter Trainium2 kernel.

Math (forward values only — the straight-through trick makes write_w equal the
hard one-hot of argmax(sim)):
  sim[b,k]   = dot(epi_keys[b,k], wk_n[b]) / (||epi_keys[b,k]|| + EPS)
  top[b]     = argmax_k sim[b,k];  best[b] = max_k sim[b,k]
  keys_new   = normalize(epi_keys) rows, except row top[b] which is the blended
               row (1-r)*epi_keys + r*write_key, normalized; r = 0.5*clip(ws,0,1)
  vals_new   = epi_vals, except row top[b] blended the same way
  age_new    = (epi_age+1) zeroed at top[b]
  str_new    = clip(0.995*str [+ ws*(1-0.995*str) at top[b]], 0.001, 1.0)

Device (8 NeuronCores, batch-parallel, 8 batches/core) streams the heavy
tensors: normalizes every epi_keys row + computes sim (the full 256MB keys
in/out), and copies epi_vals to vals_new (256MB in/out) via DRAM->DRAM DMA.
Host does the O(B*D) fix-up of the single written row per batch plus the tiny
(B,K) age/strength maps.
"""

import os
from contextlib import ExitStack

import numpy as np

EPS = 1e-6
B, K, D = 64, 2048, 512
M = 8            # cores
BPC = B // M     # batches per core
P = 128          # partitions
TPB = K // P     # k-tiles per batch

_PROGRAM = None
LAST_RESULT = None


def _emit_body(nc, tc, mybir, f32, keys_in, vals_in, wbc,
               keys_out, vals_out, sim_out):
    with ExitStack() as ctx:
        kpool = ctx.enter_context(tc.tile_pool(name="kpool", bufs=24))
        spool = ctx.enter_context(tc.tile_pool(name="spool", bufs=3))
        wpool = ctx.enter_context(tc.tile_pool(name="wpool", bufs=2))
        cpool = ctx.enter_context(tc.tile_pool(name="cpool", bufs=3))
        simpool = ctx.enter_context(tc.tile_pool(name="simpool", bufs=1))

        # Pass-through copy of epi_vals, DRAM->DRAM, in 4MB chunks.
        n_chunks = 8
        rpc = BPC * K // n_chunks
        for c in range(n_chunks):
            nc.sync.dma_start(
                out=vals_out[c * rpc : (c + 1) * rpc, :],
                in_=vals_in[c * rpc : (c + 1) * rpc, :],
            )

        sim_sb = simpool.tile([P, BPC * TPB], f32, name="sim_sb", tag="sim_sb")

        for b in range(BPC):
            wt = wpool.tile([P, D], f32, name="wt", tag="wt")
            nc.sync.dma_start(wt[:], wbc[b])
            n2 = cpool.tile([P, TPB], f32, name="n2", tag="n2")
            dot = cpool.tile([P, TPB], f32, name="dot", tag="dot")
            kts = []
            for t in range(TPB):
                r0 = b * K + t * P
                kt = kpool.tile([P, D], f32, name="kt", tag="kt")
                nc.sync.dma_start(kt[:], keys_in[r0 : r0 + P, :])
                # ACT: norm2 per row (square + accumulate along free axis)
                sq = spool.tile([P, D], f32, name="sq", tag="sq")
                nc.scalar.activation(
                    sq[:],
                    kt[:],
                    mybir.ActivationFunctionType.Square,
                    accum_out=n2[:, t : t + 1],
                )
                # DVE: dot per row with the broadcast write-key
                # (tensor_tensor_reduce crashes the runtime; this
                # InstTensorScalarPtr form computes kt*wt with accum-sum)
                prod = spool.tile([P, D], f32, name="prod", tag="prod")
                nc.vector.scalar_tensor_tensor(
                    out=prod[:],
                    in0=kt[:],
                    scalar=1.0,
                    in1=wt[:],
                    op0=mybir.AluOpType.mult,
                    op1=mybir.AluOpType.mult,
                    accum_out=dot[:, t : t + 1],
                )
                kts.append((kt, r0))

            # inv = 1 / (sqrt(n2) + EPS), batched over the 16 tile columns
            sg = cpool.tile([P, TPB], f32, name="sg", tag="sg")
            nc.scalar.activation(sg[:], n2[:], mybir.ActivationFunctionType.Sqrt)
            nc.vector.tensor_scalar_add(sg[:], sg[:], EPS)
            inv = cpool.tile([P, TPB], f32, name="inv", tag="inv")
            nc.vector.reciprocal(inv[:], sg[:])
            nc.vector.tensor_mul(
                sim_sb[:, b * TPB : (b + 1) * TPB], dot[:], inv[:]
            )

            for t, (kt, r0) in enumerate(kts):
                nc.vector.tensor_scalar_mul(kt[:], kt[:], inv[:, t : t + 1])
                nc.sync.dma_start(keys_out[r0 : r0 + P, :], kt[:])

        nc.sync.dma_start(sim_out[:], sim_sb[:])


def _build_program(reps=1):
    import concourse.bacc as bacc
    import concourse.mybir as mybir
    import concourse.tile as tile

    f32 = mybir.dt.float32
    nc = bacc.Bacc(
        "TRN2",
        target_bir_lowering=False,
        debug=False,
        num_devices=M,
    )

    keys_in = nc.dram_tensor("keys_in", [BPC * K, D], f32, kind="ExternalInput").ap()
    vals_in = nc.dram_tensor("vals_in", [BPC * K, D], f32, kind="ExternalInput").ap()
    wbc = nc.dram_tensor("wbc", [BPC, P, D], f32, kind="ExternalInput").ap()
    keys_out = nc.dram_tensor("keys_out", [BPC * K, D], f32, kind="ExternalOutput").ap()
    vals_out = nc.dram_tensor("vals_out", [BPC * K, D], f32, kind="ExternalOutput").ap()
    sim_out = nc.dram_tensor("sim_out", [P, BPC * TPB], f32, kind="ExternalOutput").ap()

    with tile.TileContext(nc) as tc:
        for _rep in range(reps):
            _emit_body(nc, tc, mybir, f32, keys_in, vals_in, wbc,
                       keys_out, vals_out, sim_out)

    nc.compile()
    return nc


def _get_program():
    global _PROGRAM
    if _PROGRAM is None:
        _PROGRAM = _build_program()
    return _PROGRAM


def kernel(**inputs):
    global LAST_RESULT
    from concourse.bass_utils import run_bass_kernel_spmd

    wk = np.asarray(inputs["write_key"], dtype=np.float32)
    wv = np.asarray(inputs["write_val"], dtype=np.float32)
    ws_raw = np.asarray(inputs["write_strength"], dtype=np.float32)
    ek = np.ascontiguousarray(np.asarray(inputs["epi_keys"], dtype=np.float32))
    ev = np.ascontiguousarray(np.asarray(inputs["epi_vals"], dtype=np.float32))
    ea = np.asarray(inputs["epi_age"], dtype=np.float32)
    es = np.asarray(inputs["epi_strength"], dtype=np.float32)

    nc = _get_program()

    wk_n = wk / (np.linalg.norm(wk, axis=-1, keepdims=True) + np.float32(EPS))
    in_maps = []
    for c in range(M):
        sl = slice(c * BPC, (c + 1) * BPC)
        in_maps.append(
            {
                "keys_in": ek[sl].reshape(BPC * K, D),
                "vals_in": ev[sl].reshape(BPC * K, D),
                "wbc": np.ascontiguousarray(
                    np.broadcast_to(wk_n[sl][:, None, :], (BPC, P, D))
                ),
            }
        )

    trace = bool(int(os.environ.get("KERNEL_TRACE", "0")))
    res = run_bass_kernel_spmd(nc, in_maps, list(range(M)), trace=trace)
    LAST_RESULT = res

    keys_new = np.empty((B, K, D), dtype=np.float32)
    vals_new = np.empty((B, K, D), dtype=np.float32)
    sim = np.empty((B, K), dtype=np.float32)
    for c in range(M):
        sl = slice(c * BPC, (c + 1) * BPC)
        keys_new[sl] = res.results[c]["keys_out"].reshape(BPC, K, D)
        vals_new[sl] = res.results[c]["vals_out"].reshape(BPC, K, D)
        # sim_out is [p, b*TPB + t] with k = t*P + p
        sim_raw = res.results[c]["sim_out"].reshape(P, BPC, TPB)
        sim[sl] = sim_raw.transpose(1, 2, 0).reshape(BPC, K)

    top = sim.argmax(axis=1)
    best_sim = sim.max(axis=1)
    slot_idx = top.astype(np.int32)

    ws = np.clip(ws_raw, 0.0, 1.0)
    r = ws * np.float32(0.5)  # (B,)
    ar = np.arange(B)

    # fix the written row of keys (blend then normalize)
    kb = (1.0 - r)[:, None] * ek[ar, top] + r[:, None] * wk
    kbn = kb / (np.linalg.norm(kb, axis=-1, keepdims=True) + np.float32(EPS))
    keys_new[ar, top] = kbn

    # fix the written row of vals
    vals_new[ar, top] = (1.0 - r)[:, None] * ev[ar, top] + r[:, None] * wv

    age_new = ea + np.float32(1.0)
    age_new[ar, top] = 0.0

    s0 = es * np.float32(0.995)
    str_new = np.clip(s0, 0.001, 1.0).astype(np.float32)
    str_new[ar, top] = np.clip(
        s0[ar, top] + ws * (np.float32(1.0) - s0[ar, top]), 0.001, 1.0
    )

    return keys_new, vals_new, age_new, str_new, slot_idx, best_sim
